# revision 46
# baseline (speedup 1.0000x reference)
"""Trainium2 Bass kernel for nn_CrossAttention (B=8, N=4096, C=512, H=8, d=64).

Math (per batch element b, handled by one NeuronCore):
    kv_j = x_j @ Wkv_j ; k_j, v_j = heads(kv_j)
    ctx_j = scale * k_jh^T v_jh            (per head, [d, d])
          = scale * Wk_jh^T (x_j^T x_j) Wv_jh     <-- Gram trick: G_j = x_j^T x_j
    s_j = softmax(ctx_j, axis over first d)
    out1 = concat_h(q1_h @ s2_h),  out2 = concat_h(q2_h @ s1_h),  q_j = heads(x_j)

Key design points (182.8us -> 113us -> ~98us history):
  * x/xT/out in HBM as fp16; G/W/T internals f32(r) except the t/Wv fp16
    head-loop (logits keep ~1e-2 headroom vs the 2e-2 tolerance).
  * G symmetric: only upper-triangular blocks on the PE; lower blocks
    mirrored with PE transposes, pipelined per block-row m with the
    evacuation copies and the T = G W m-loop so the seam never idles the PE.
  * PE p-state: the tensor engine clocks 0.65/1.2/2.4 GHz and needs ~3us of
    continuous work to reach max speed. Warmup matmuls cover engine start,
    and the schedule interleaves stream/finish/ctx work to avoid gaps.
  * x loads use "(p a) c" grouping: each partition holds 4 consecutive
    rows = 4KB contiguous HBM runs (23 B/ns/engine vs 17 at 1KB). Valid
    because G = x^T x is token-permutation invariant.
  * Softmax: K and V share x so scaled logits spread ~N(0,24) -- max
    subtraction is required (f32 exp overflows without it). Batched per-
    parity 3D max-reduce, broadcast subtract, exp, sum-reduce, reciprocal;
    head-pair PE transposes land both s_sb quadrants via in-lane copies.
  * The q@s passes compute outT (s stationary, xT moving, 512 tokens per
    matmul); PSUM->SBUF staging alternates DVE/ACT whole-tile copies.
    out2 drains via gpsimd SWDGE (Pool engine is idle then), out1 via the
    sync queue -- DMA triggers cost the issuing engine's sequencer ~0.6us
    each, so they must ride engines with no compute duties.

Sharding: batch b -> core b (8 cores, no collectives).
"""

import numpy as np
from contextlib import ExitStack

import concourse.bass as bass
import concourse.tile as tile
from concourse import bacc, mybir, masks
from concourse.bass_utils import run_bass_kernel_spmd

F32 = mybir.dt.float32
F32R = mybir.dt.float32r
F16 = mybir.dt.float16

B, N, C = 8, 4096, 512
H, D = 8, 64
SCALE = float(D) ** -0.5
TT = 128            # token tile
NTT = N // TT       # 32 token tiles
CK = C // 128       # 4 chan blocks
NG = N // 512       # 8 groups of 512 tokens

_CACHE = {}


def _emit(tc, io):
    nc = tc.nc
    x_d = [io["x1"], io["x2"]]
    w_d = [io["Wkv1"], io["Wkv2"]]
    o_d = [io["out1"], io["out2"]]
    xT_d = [io["x1T"], io["x2T"]]

    ctx = ExitStack()
    with ctx:
        pers = ctx.enter_context(tc.tile_pool(name="pers", bufs=1))
        xin = ctx.enter_context(tc.tile_pool(name="xin", bufs=6))
        tsb = ctx.enter_context(tc.tile_pool(name="tsb", bufs=2))
        smp = ctx.enter_context(tc.tile_pool(name="smp", bufs=2))
        outp = ctx.enter_context(tc.tile_pool(name="outp", bufs=10))

        # PSUM budget is 8 banks: scr(1) + ctx(1) + t(2) + big(4) = 8 while
        # streaming; big(4) closes before the out pool (4) opens.
        scr_ps = ctx.enter_context(tc.tile_pool(name="scr_ps", bufs=1, space="PSUM"))
        ctx_pool = ctx.enter_context(tc.tile_pool(name="ctx_ps", bufs=1, space="PSUM"))
        t_pool = ctx.enter_context(tc.tile_pool(name="t_ps", bufs=2, space="PSUM"))
        sc_big = ExitStack()
        big_ps = sc_big.enter_context(tc.tile_pool(name="big_ps", bufs=1, space="PSUM"))

        # ---- persistent SBUF ----
        # W ships from the host as fp16 so it rides the scalar HWDGE queue
        # as 2-byte traffic (SWDGE f32 loads get starved behind the HWDGE
        # x/xT streams for 30+us); G also lives as fp16. t = G W in fp16
        # costs ~0.3% extra softmax error against the 2e-2 budget.
        w16 = pers.tile([128, 2 * CK * 1024], F16)

        def wv(jw, c0, c1):  # slice of W tensor jw in fp16 column units
            return w16[:, jw * 4096 + c0 : jw * 4096 + c1]
        # xT (fp16): cols [j*16384 + c*4096 + tok] = x_j[tok, 128c + p]
        xT_sb = pers.tile([128, 2 * CK * N], F16)
        g_sb = pers.tile([128, 2 * CK * 512], F16)
        # S blocks (fp16): tensor j, head-pair k at cols [j*1024 + 128k : +128];
        # quadrants [0:64,0:64]=s_{2k}, [64:128,64:128]=s_{2k+1}, off-diag 0.
        s_sb = pers.tile([128, 2 * 1024], F16)
        nc.gpsimd.memset(s_sb[:], 0.0)
        # G2's last two groups get dedicated tiles loaded early on the
        # scalar queue: at the tail of sync's 8MB x-stream they would
        # otherwise arrive just as the PE needs them (a ~2.6us stall), and
        # pool-slot WAR coupling would head-of-line block scalar if they
        # shared the xin pool.
        xg_tail = pers.tile([128, 2 * 2048], F16)
        ident_f = pers.tile([128, 128], F32)
        masks.make_identity(nc, ident_f[:])
        ident16 = pers.tile([128, 128], F16)
        nc.vector.tensor_copy(ident16[:], ident_f[:])

        # PE p-state warmup: dummy f32 matmuls (each lowers to 2 half-rate
        # passes) between engine start and the first x group's arrival so
        # real matmuls run at full clock from the first tile.
        warm_ps = scr_ps.tile([128, 128], F32, name="warm", tag="scr")
        with tc.high_priority():
            for _ in range(4):
                nc.tensor.matmul(warm_ps[:], ident_f[:], ident_f[:],
                                 start=True, stop=True)

        # ---- DMA queue policy ----
        #   sync   (HWDGE): x group loads, then xT1, then all out writes
        #   scalar (HWDGE): first half-group, xT2
        def load_w(jw):
            for k in range(CK):
                nc.scalar.dma_start(
                    w16[:, jw * 4096 + k * 1024 : jw * 4096 + (k + 1) * 1024],
                    w_d[jw][128 * k : 128 * (k + 1), :],
                )

        def load_xT(j, deng):
            for c in range(CK):
                off = j * 16384 + c * 4096
                deng.dma_start(
                    xT_sb[:, off : off + 4096],
                    xT_d[j][128 * c : 128 * (c + 1), :],
                )

        ncopy = [0]

        def eng_copy(dst, src_):
            ncopy[0] += 1
            if ncopy[0] % 2:
                nc.vector.tensor_copy(dst, src_)
            else:
                nc.scalar.activation(dst, src_, mybir.ActivationFunctionType.Copy)

        def stream_g(j, g_ps, g_range, gates=None):
            """Stream x_j in 512-token groups; accumulate upper-triangular
            blocks of G_j = x_j^T x_j in PSUM. Sub-tile s holds tokens
            {4p+s}: any distinct 128-token set works for the Gram sum."""
            for g in g_range:
                if j == 1 and g >= NG - 2:
                    xga = xg_tail[:, 2048 * (g - NG + 2) : 2048 * (g - NG + 3)]
                    nc.scalar.dma_start(
                        xga,
                        x_d[j][512 * g : 512 * (g + 1), :].rearrange(
                            "(p a) c -> p (a c)", p=128
                        ),
                    )
                else:
                    xg = xin.tile([128, 2048], F16, name="xg", tag="xt")
                    xga = xg[:]
                    if j == 0 and g == 0:
                        nc.sync.dma_start(
                            xg[:, 0:1024],
                            x_d[0][0:256, :].rearrange("(p a) c -> p (a c)", p=128),
                        )
                        nc.scalar.dma_start(
                            xg[:, 1024:2048],
                            x_d[0][256:512, :].rearrange("(p a) c -> p (a c)", p=128),
                        )
                    else:
                        nc.sync.dma_start(
                            xg[:],
                            x_d[j][512 * g : 512 * (g + 1), :].rearrange(
                                "(p a) c -> p (a c)", p=128
                            ),
                        )
                if gates and g in gates:
                    # Poke one column of each pending bulk-load target from
                    # this group tile: the bulk DMAs WAW-wait on the pokes,
                    # keeping them out of earlier streaming windows.
                    for dst in gates[g]:
                        nc.vector.tensor_copy(dst, xga[:, 0:1])
                for s in range(4):
                    xt = xga[:, 512 * s : 512 * (s + 1)]
                    for m in range(CK):
                        nm = 512 - 128 * m
                        nc.tensor.matmul(
                            g_ps[m][:, 0:nm],
                            xt[:, 128 * m : 128 * (m + 1)],
                            xt[:, 128 * m : 512],
                            start=(g == 0 and s == 0),
                            stop=(g == NG - 1 and s == 3),
                        )

        def g_evac(j, g_ps):
            # upper blocks (m,k), k>=m: g_ps[m][:, 128(k-m):...] -> block cols
            for m in range(CK):
                nm = 512 - 128 * m
                h1 = 128 * ((CK - m + 1) // 2)
                base = j * 2048 + 512 * m + 128 * m
                eng_copy(g_sb[:, base : base + h1], g_ps[m][:, 0:h1])
                if h1 < nm:
                    eng_copy(g_sb[:, base + h1 : j * 2048 + 512 * (m + 1)],
                             g_ps[m][:, h1:nm])

        def mirror_t_ctx(j, ctx_ps):
            """Per block-row m: mirror the lower blocks (k,m) k>m from row m's
            uppers, then run the T = G W m-loop for column m (all its inputs
            are ready exactly then), evacuate t to fp16, pipelined with the
            next m. Then the fp16 head-loop ctxT_h = Wv_h^T T_h."""
            ctx_t = ctx_pool.tile([64, 512], F32, name=f"ctx{j}", tag="ctx")
            ctx_ps[(j, 0)] = ctx_t[0:64, 0:256]
            ctx_ps[(j, 1)] = ctx_t[0:64, 256:512]
            t_sb = tsb.tile([128, 2048], F16, name="tsb", tag="tsb")
            for m in range(CK):
                nmir = CK - 1 - m
                if nmir:
                    tp = scr_ps.tile([128, 128 * nmir], F16, name="mir", tag="scr")
                    for i, k in enumerate(range(m + 1, CK)):
                        # block (k, m) = block (m, k)^T ; source row m upper
                        nc.tensor.transpose(
                            tp[:, 128 * i : 128 * (i + 1)],
                            g_sb[:, j * 2048 + 512 * m + 128 * k : j * 2048 + 512 * m + 128 * (k + 1)],
                            ident16[:],
                        )
                    for i, k in enumerate(range(m + 1, CK)):
                        eng_copy(
                            g_sb[:, j * 2048 + 512 * k + 128 * m : j * 2048 + 512 * k + 128 * (m + 1)],
                            tp[:, 128 * i : 128 * (i + 1)],
                        )
                t_ps = t_pool.tile([128, 512], F32, name=f"t{m}", tag="t")
                for k in range(CK):
                    nc.tensor.matmul(
                        t_ps[:],
                        g_sb[:, j * 2048 + 512 * k + 128 * m : j * 2048 + 512 * k + 128 * (m + 1)],
                        wv(j, 1024 * k, 1024 * k + 512),
                        start=(k == 0),
                        stop=(k == CK - 1),
                    )
                eng_copy(t_sb[:, 512 * m : 512 * (m + 1)], t_ps[:])
            for h in (0, 2, 4, 6, 1, 3, 5, 7):
                cps = ctx_ps[(j, h % 2)]
                q = h // 2
                for k in range(CK):
                    nc.tensor.matmul(
                        cps[:, 64 * q : 64 * (q + 1)],
                        wv(j, 1024 * k + 512 + 64 * h, 1024 * k + 512 + 64 * (h + 1)),
                        t_sb[:, 512 * k + 64 * h : 512 * k + 64 * (h + 1)],
                        start=(k == 0),
                        stop=(k == CK - 1),
                    )

        def softmax(j, ctx_ps):
            """Batched softmax: K and V share x, so ctx has a correlated mean
            component and scaled logits spread ~N(0,24) - max-subtraction is
            required (tails overflow f32 exp). Per-head maxes come from one
            3D reduce per parity; the subtract is a broadcast
            scalar_tensor_tensor; exp/sums/reciprocal are batched; fp16
            head-pairs go through PE pair transposes with in-lane quadrant
            copies into s_sb."""
            nmax = smp.tile([64, 8], F32, name="nmax", tag="nmax")
            nmaxs = smp.tile([64, 8], F32, name="nmaxs", tag="nmaxs")
            z_sb = smp.tile([64, 512], F32, name="zsb", tag="zsb")
            e_sb = smp.tile([64, 512], F32, name="esb", tag="esb")
            for par in range(2):
                nc.vector.tensor_reduce(
                    nmax[:, 4 * par : 4 * par + 4],
                    ctx_ps[(j, par)].rearrange("p (q c) -> p q c", c=64),
                    mybir.AxisListType.X, mybir.AluOpType.max, negate=True,
                )
            nc.vector.tensor_scalar_mul(nmaxs[:], nmax[:], SCALE)
            for par in range(2):
                nc.vector.scalar_tensor_tensor(
                    z_sb[:, 256 * par : 256 * (par + 1)].rearrange("p (q c) -> p q c", c=64),
                    ctx_ps[(j, par)].rearrange("p (q c) -> p q c", c=64),
                    SCALE,
                    nmaxs[:, 4 * par : 4 * par + 4].unsqueeze(2).broadcast_to([64, 4, 64]),
                    mybir.AluOpType.mult,
                    mybir.AluOpType.add,
                )
                nc.scalar.activation(
                    e_sb[:, 256 * par : 256 * (par + 1)],
                    z_sb[:, 256 * par : 256 * (par + 1)],
                    mybir.ActivationFunctionType.Exp,
                )
            ssum = smp.tile([64, 8], F32, name="ssum", tag="ssum")
            for par in range(2):
                nc.vector.tensor_reduce(
                    ssum[:, 4 * par : 4 * par + 4],
                    e_sb[:, 256 * par : 256 * (par + 1)].rearrange("p (q c) -> p q c", c=64),
                    mybir.AxisListType.X, mybir.AluOpType.add,
                )
            rec = smp.tile([64, 8], F32, name="rec", tag="rec")
            nc.vector.reciprocal(rec[:], ssum[:])
            # sT pair q: cols [128q:128q+64] = head 2q, [+64:+128] = head 2q+1
            sT = smp.tile([64, 512], F16, name="sT", tag="sT")
            for q in range(4):
                for par in range(2):
                    nc.vector.tensor_scalar_mul(
                        sT[:, 128 * q + 64 * par : 128 * q + 64 * (par + 1)],
                        e_sb[:, 256 * par + 64 * q : 256 * par + 64 * (q + 1)],
                        rec[:, 4 * par + q : 4 * par + q + 1],
                    )
                tp = scr_ps.tile([128, 64], F16, name="sps", tag="scr")
                nc.tensor.transpose(tp[:], sT[:, 128 * q : 128 * (q + 1)], ident16[0:64, 0:64])
                eng_copy(s_sb[0:64, j * 1024 + 128 * q : j * 1024 + 128 * q + 64], tp[0:64, :])
                eng_copy(s_sb[64:128, j * 1024 + 128 * q + 64 : j * 1024 + 128 * (q + 1)], tp[64:128, :])
            if j == 0 and "dbg_e" in io:
                nc.sync.dma_start(io["dbg_e"], e_sb[:])
                nc.sync.dma_start(io["dbg_sum"][:, 0:8], ssum[:])
                nc.sync.dma_start(io["dbg_sum"][:, 8:16], rec[:])
                nc.sync.dma_start(io["dbg_sT"], sT[:])

        opair = {}
        nstg = [0]

        def stage_copy(dst, src_):
            # Whole-tile PSUM->SBUF staging alternating DVE/ACT: combined
            # PSUM-read rate ~1.1TB/s keeps pace with the a-pass matmuls.
            nstg[0] += 1
            if nstg[0] % 2:
                nc.vector.tensor_copy(dst, src_)
            else:
                nc.scalar.activation(dst, src_, mybir.ActivationFunctionType.Copy)

        def emit_outT(j, k, g, o_ps):
            """Stage a finished outT tile (f32 PSUM -> fp16 SBUF); DMA every
            completed pair of token groups with one 256KB transfer. The final
            two groups of the very last a1 block go out singly so the drain
            tail is as short as possible."""
            if j == 0 and k == CK - 1 and g >= 6:
                o_sb = outp.tile([128, 1024], F16, name="osb", tag="osb")
                stage_copy(o_sb[:, 0:512], o_ps[:])
                nc.sync.dma_start(
                    o_d[j][128 * k : 128 * (k + 1), 512 * g : 512 * (g + 1)],
                    o_sb[:, 0:512],
                )
                return
            gg, half = divmod(g, 2)
            key = (j, k, gg)
            if key not in opair:
                opair[key] = outp.tile([128, 1024], F16, name="osb", tag="osb")
            o_sb = opair[key]
            c0 = 512 * half
            stage_copy(o_sb[:, c0 : c0 + 512], o_ps[:])
            if half == 1:
                o_sb = opair.pop(key)
                (nc.sync if j == 0 else nc.gpsimd).dma_start(
                    o_d[j][128 * k : 128 * (k + 1), 1024 * gg : 1024 * (gg + 1)],
                    o_sb[:, 0:1024],
                )

        def a_pass(jq, js, out_pool):
            """outT_{jq}[chout, tok] = sum_chin S_{js}[chin, chout] q_{jq}[tok, chin].
            s stationary, xT moving: 512 tokens per matmul."""
            for k in range(CK):
                for g in range(N // 512):
                    o_ps = out_pool.tile([128, 512], F32, name=f"o{jq}ps", tag="ops")
                    nc.tensor.matmul(
                        o_ps[:],
                        s_sb[:, js * 1024 + 128 * k : js * 1024 + 128 * (k + 1)],
                        xT_sb[:, jq * 16384 + k * 4096 + 512 * g : jq * 16384 + k * 4096 + 512 * (g + 1)],
                        start=True,
                        stop=True,
                    )
                    emit_outT(jq, k, g, o_ps)

        # ---------------- schedule ----------------
        ctx_ps = {}
        g1 = [big_ps.tile([128, 512], F32, name=f"g1{m}", tag=f"big{m}") for m in range(CK)]
        # Both W tensors gated just after the stream starts: they ride the
        # scalar queue 7-17us, done well before xT contention and in time
        # for the m-loops (a late W turns into multi-us engine-queue stalls
        # behind the wv16 casts).
        stream_g(0, g1, list(range(NG)),
                 gates={1: [wv(0, 1024 * k, 1024 * k + 1) for k in range(CK)],
                        2: [xg_tail[:, 0:1], xg_tail[:, 2048:2049]],
                        3: [wv(1, 1024 * k, 1024 * k + 1) for k in range(CK)]})
        load_w(0)
        load_w(1)
        g_evac(0, g1)

        # G2 head groups cover the evac/mirror/t_ctx(0) seam on the PE: their
        # matmuls only WAR-wait on the g1 banks being copied out.
        g2 = [big_ps.tile([128, 512], F32, name=f"g2{m}", tag=f"big{m}") for m in range(CK)]
        stream_g(1, g2, list(range(0, 2)))

        mirror_t_ctx(0, ctx_ps)                              # T(1), ctxT(1)

        stream_g(1, g2, list(range(2, 4)),
                 gates={3: [xT_sb[:, 16384 + c * 4096 : 16385 + c * 4096] for c in range(CK)]})
        load_xT(1, nc.scalar)
        with tc.high_priority():
            softmax(0, ctx_ps)                               # s1
        stream_g(1, g2, list(range(4, NG)),
                 gates={5: [xT_sb[:, c * 4096 : c * 4096 + 1] for c in range(CK)]})
        load_xT(0, nc.sync)
        g_evac(1, g2)
        sc_big.close()

        mirror_t_ctx(1, ctx_ps)                              # T(2), ctxT(2)

        sc_out = ExitStack()
        out_pool = sc_out.enter_context(tc.tile_pool(name="out_ps", bufs=4, space="PSUM"))
        a_pass(1, 0, out_pool)                               # out2 = q2 @ s1
        with tc.high_priority():
            softmax(1, ctx_ps)                               # s2
        a_pass(0, 1, out_pool)                               # out1 = q1 @ s2
        sc_out.close()

        if "dbg_xT" in io:
            nc.sync.dma_start(io["dbg_xT"], xT_sb[:])
            nc.sync.dma_start(io["dbg_s"], s_sb[:])
            nc.sync.dma_start(io["dbg_g"], g_sb[:])


def _build():
    if "nc" in _CACHE:
        return _CACHE["nc"]
    nc = bacc.Bacc("TRN2", target_bir_lowering=False, debug=False, num_devices=B)
    io = {
        "x1": nc.dram_tensor("x1", [N, C], F16, kind="ExternalInput").ap(),
        "x2": nc.dram_tensor("x2", [N, C], F16, kind="ExternalInput").ap(),
        "x1T": nc.dram_tensor("x1T", [C, N], F16, kind="ExternalInput").ap(),
        "x2T": nc.dram_tensor("x2T", [C, N], F16, kind="ExternalInput").ap(),
        "Wkv1": nc.dram_tensor("Wkv1", [C, 2 * C], F16, kind="ExternalInput").ap(),
        "Wkv2": nc.dram_tensor("Wkv2", [C, 2 * C], F16, kind="ExternalInput").ap(),
        "out1": nc.dram_tensor("out1", [C, N], F16, kind="ExternalOutput").ap(),
        "out2": nc.dram_tensor("out2", [C, N], F16, kind="ExternalOutput").ap(),
    }

    with tile.TileContext(nc) as tc:
        _emit(tc, io)
    nc.compile()
    _CACHE["nc"] = nc
    return nc


def kernel(x1, x2, Wkv1, Wkv2):
    x1 = np.ascontiguousarray(np.asarray(x1, dtype=np.float32).astype(np.float16))
    x2 = np.ascontiguousarray(np.asarray(x2, dtype=np.float32).astype(np.float16))
    Wkv1 = np.ascontiguousarray(np.asarray(Wkv1, dtype=np.float32).astype(np.float16))
    Wkv2 = np.ascontiguousarray(np.asarray(Wkv2, dtype=np.float32).astype(np.float16))

    nc = _build()
    in_maps = [
        {
            "x1": x1[b], "x2": x2[b],
            "x1T": np.ascontiguousarray(x1[b].T),
            "x2T": np.ascontiguousarray(x2[b].T),
            "Wkv1": Wkv1, "Wkv2": Wkv2,
        }
        for b in range(B)
    ]
    res = run_bass_kernel_spmd(nc, in_maps, list(range(B))).results
    out1 = np.stack([res[b]["out1"].T for b in range(B)]).astype(np.float32)
    out2 = np.stack([res[b]["out2"].T for b in range(B)]).astype(np.float32)
    return out1, out2


if __name__ == "__main__":
    rng = np.random.default_rng(0)
    o1, o2 = kernel(
        rng.standard_normal((B, N, C), dtype=np.float32),
        rng.standard_normal((B, N, C), dtype=np.float32),
        rng.standard_normal((C, 2 * C), dtype=np.float32) * C**-0.5,
        rng.standard_normal((C, 2 * C), dtype=np.float32) * C**-0.5,
    )
    print(o1.shape, o2.shape)


# revision 47
# speedup vs baseline: 1.0566x; 1.0566x over previous
"""Trainium2 Bass kernel for nn_CrossAttention (B=8, N=4096, C=512, H=8, d=64).

Math (per batch element b, handled by one NeuronCore):
    kv_j = x_j @ Wkv_j ; k_j, v_j = heads(kv_j)
    ctx_j = scale * k_jh^T v_jh            (per head, [d, d])
          = scale * Wk_jh^T (x_j^T x_j) Wv_jh     <-- Gram trick: G_j = x_j^T x_j
    s_j = softmax(ctx_j, axis over first d)
    out1 = concat_h(q1_h @ s2_h),  out2 = concat_h(q2_h @ s1_h),  q_j = heads(x_j)

Key design points (182.8us -> 113us -> ~98us history):
  * x/xT/out in HBM as fp16; G/W/T internals f32(r) except the t/Wv fp16
    head-loop (logits keep ~1e-2 headroom vs the 2e-2 tolerance).
  * G symmetric: only upper-triangular blocks on the PE; lower blocks
    mirrored with PE transposes, pipelined per block-row m with the
    evacuation copies and the T = G W m-loop so the seam never idles the PE.
  * PE p-state: the tensor engine clocks 0.65/1.2/2.4 GHz and needs ~3us of
    continuous work to reach max speed. Warmup matmuls cover engine start,
    and the schedule interleaves stream/finish/ctx work to avoid gaps.
  * x loads use "(p a) c" grouping: each partition holds 4 consecutive
    rows = 4KB contiguous HBM runs (23 B/ns/engine vs 17 at 1KB). Valid
    because G = x^T x is token-permutation invariant.
  * Softmax: K and V share x so scaled logits spread ~N(0,24) -- max
    subtraction is required (f32 exp overflows without it). Batched per-
    parity 3D max-reduce, broadcast subtract, exp, sum-reduce, reciprocal;
    head-pair PE transposes land both s_sb quadrants via in-lane copies.
  * The q@s passes compute outT (s stationary, xT moving, 512 tokens per
    matmul); PSUM->SBUF staging alternates DVE/ACT whole-tile copies.
    out2 drains via gpsimd SWDGE (Pool engine is idle then), out1 via the
    sync queue -- DMA triggers cost the issuing engine's sequencer ~0.6us
    each, so they must ride engines with no compute duties.

Sharding: batch b -> core b (8 cores, no collectives).
"""

import numpy as np
from contextlib import ExitStack

import concourse.bass as bass
import concourse.tile as tile
from concourse import bacc, mybir, masks
from concourse.bass_utils import run_bass_kernel_spmd

F32 = mybir.dt.float32
F32R = mybir.dt.float32r
F16 = mybir.dt.float16

B, N, C = 8, 4096, 512
H, D = 8, 64
SCALE = float(D) ** -0.5
TT = 128            # token tile
NTT = N // TT       # 32 token tiles
CK = C // 128       # 4 chan blocks
NG = N // 512       # 8 groups of 512 tokens

_CACHE = {}


def _emit(tc, io):
    nc = tc.nc
    x_d = [io["x1"], io["x2"]]
    w_d = [io["Wkv1"], io["Wkv2"]]
    o_d = [io["out1"], io["out2"]]
    xT_d = [io["x1T"], io["x2T"]]

    ctx = ExitStack()
    with ctx:
        pers = ctx.enter_context(tc.tile_pool(name="pers", bufs=1))
        xin = ctx.enter_context(tc.tile_pool(name="xin", bufs=6))
        tsb = ctx.enter_context(tc.tile_pool(name="tsb", bufs=2))
        smp = ctx.enter_context(tc.tile_pool(name="smp", bufs=2))
        outp = ctx.enter_context(tc.tile_pool(name="outp", bufs=10))

        # PSUM budget is 8 banks: scr(1) + ctx(1) + t(2) + big(4) = 8 while
        # streaming; big(4) closes before the out pool (4) opens.
        scr_ps = ctx.enter_context(tc.tile_pool(name="scr_ps", bufs=1, space="PSUM"))
        ctx_pool = ctx.enter_context(tc.tile_pool(name="ctx_ps", bufs=1, space="PSUM"))
        t_pool = ctx.enter_context(tc.tile_pool(name="t_ps", bufs=2, space="PSUM"))
        sc_big = ExitStack()
        big_ps = sc_big.enter_context(tc.tile_pool(name="big_ps", bufs=1, space="PSUM"))

        # ---- persistent SBUF ----
        # W ships from the host as fp16 so it rides the scalar HWDGE queue
        # as 2-byte traffic (SWDGE f32 loads get starved behind the HWDGE
        # x/xT streams for 30+us); G also lives as fp16. t = G W in fp16
        # costs ~0.3% extra softmax error against the 2e-2 budget.
        w16 = pers.tile([128, 2 * CK * 1024], F16)

        def wv(jw, c0, c1):  # slice of W tensor jw in fp16 column units
            return w16[:, jw * 4096 + c0 : jw * 4096 + c1]
        # xT (fp16): cols [j*16384 + c*4096 + tok] = x_j[tok, 128c + p]
        xT_sb = pers.tile([128, 2 * CK * N], F16)
        g_sb = pers.tile([128, 2 * CK * 512], F16)
        # S blocks (fp16): tensor j, head-pair k at cols [j*1024 + 128k : +128];
        # quadrants [0:64,0:64]=s_{2k}, [64:128,64:128]=s_{2k+1}, off-diag 0.
        s_sb = pers.tile([128, 2 * 1024], F16)
        nc.gpsimd.memset(s_sb[:], 0.0)
        ident_f = pers.tile([128, 128], F32)
        masks.make_identity(nc, ident_f[:])
        ident16 = pers.tile([128, 128], F16)
        nc.vector.tensor_copy(ident16[:], ident_f[:])

        # PE p-state warmup: dummy f32 matmuls (each lowers to 2 half-rate
        # passes) between engine start and the first x group's arrival so
        # real matmuls run at full clock from the first tile.
        warm_ps = scr_ps.tile([128, 128], F32, name="warm", tag="scr")
        with tc.high_priority():
            for _ in range(4):
                nc.tensor.matmul(warm_ps[:], ident_f[:], ident_f[:],
                                 start=True, stop=True)

        # ---- DMA queue policy ----
        #   sync   (HWDGE): x group loads, then xT1, then all out writes
        #   scalar (HWDGE): first half-group, xT2
        def load_w(jw):
            for k in range(CK):
                nc.scalar.dma_start(
                    w16[:, jw * 4096 + k * 1024 : jw * 4096 + (k + 1) * 1024],
                    w_d[jw][128 * k : 128 * (k + 1), :],
                )

        def load_xT(j, deng):
            for c in range(CK):
                off = j * 16384 + c * 4096
                deng.dma_start(
                    xT_sb[:, off : off + 4096],
                    xT_d[j][128 * c : 128 * (c + 1), :],
                )

        ncopy = [0]

        def eng_copy(dst, src_):
            ncopy[0] += 1
            if ncopy[0] % 2:
                nc.vector.tensor_copy(dst, src_)
            else:
                nc.scalar.activation(dst, src_, mybir.ActivationFunctionType.Copy)

        def stream_g(j, g_ps, g_range, gates=None):
            """Stream x_j in 512-token groups; accumulate upper-triangular
            blocks of G_j = x_j^T x_j in PSUM. Sub-tile s holds tokens
            {4p+s}: any distinct 128-token set works for the Gram sum."""
            for g in g_range:
                xg = xin.tile([128, 2048], F16, name="xg", tag="xt")
                if j == 0 and g == 0:
                    nc.sync.dma_start(
                        xg[:, 0:1024],
                        x_d[0][0:256, :].rearrange("(p a) c -> p (a c)", p=128),
                    )
                    nc.scalar.dma_start(
                        xg[:, 1024:2048],
                        x_d[0][256:512, :].rearrange("(p a) c -> p (a c)", p=128),
                    )
                else:
                    nc.sync.dma_start(
                        xg[:],
                        x_d[j][512 * g : 512 * (g + 1), :].rearrange(
                            "(p a) c -> p (a c)", p=128
                        ),
                    )
                if gates and g in gates:
                    # Poke one column of each pending bulk-load target from
                    # this group tile: the bulk DMAs WAW-wait on the pokes,
                    # keeping them out of earlier streaming windows.
                    for dst in gates[g]:
                        nc.vector.tensor_copy(dst, xg[:, 0:1])
                for s in range(4):
                    xt = xg[:, 512 * s : 512 * (s + 1)]
                    for m in range(CK):
                        nm = 512 - 128 * m
                        nc.tensor.matmul(
                            g_ps[m][:, 0:nm],
                            xt[:, 128 * m : 128 * (m + 1)],
                            xt[:, 128 * m : 512],
                            start=(g == 0 and s == 0),
                            stop=(g == NG - 1 and s == 3),
                        )

        def g_evac(j, g_ps):
            # upper blocks (m,k), k>=m: g_ps[m][:, 128(k-m):...] -> block cols
            for m in range(CK):
                nm = 512 - 128 * m
                h1 = 128 * ((CK - m + 1) // 2)
                base = j * 2048 + 512 * m + 128 * m
                eng_copy(g_sb[:, base : base + h1], g_ps[m][:, 0:h1])
                if h1 < nm:
                    eng_copy(g_sb[:, base + h1 : j * 2048 + 512 * (m + 1)],
                             g_ps[m][:, h1:nm])

        def mirror_t_ctx(j, ctx_ps):
            """Per block-row m: mirror the lower blocks (k,m) k>m from row m's
            uppers, then run the T = G W m-loop for column m (all its inputs
            are ready exactly then), evacuate t to fp16, pipelined with the
            next m. Then the fp16 head-loop ctxT_h = Wv_h^T T_h."""
            ctx_t = ctx_pool.tile([64, 512], F32, name=f"ctx{j}", tag="ctx")
            ctx_ps[(j, 0)] = ctx_t[0:64, 0:256]
            ctx_ps[(j, 1)] = ctx_t[0:64, 256:512]
            t_sb = tsb.tile([128, 2048], F16, name="tsb", tag="tsb")
            for m in range(CK):
                nmir = CK - 1 - m
                if nmir:
                    tp = scr_ps.tile([128, 128 * nmir], F16, name="mir", tag="scr")
                    for i, k in enumerate(range(m + 1, CK)):
                        # block (k, m) = block (m, k)^T ; source row m upper
                        nc.tensor.transpose(
                            tp[:, 128 * i : 128 * (i + 1)],
                            g_sb[:, j * 2048 + 512 * m + 128 * k : j * 2048 + 512 * m + 128 * (k + 1)],
                            ident16[:],
                        )
                    for i, k in enumerate(range(m + 1, CK)):
                        eng_copy(
                            g_sb[:, j * 2048 + 512 * k + 128 * m : j * 2048 + 512 * k + 128 * (m + 1)],
                            tp[:, 128 * i : 128 * (i + 1)],
                        )
                t_ps = t_pool.tile([128, 512], F32, name=f"t{m}", tag="t")
                for k in range(CK):
                    nc.tensor.matmul(
                        t_ps[:],
                        g_sb[:, j * 2048 + 512 * k + 128 * m : j * 2048 + 512 * k + 128 * (m + 1)],
                        wv(j, 1024 * k, 1024 * k + 512),
                        start=(k == 0),
                        stop=(k == CK - 1),
                    )
                eng_copy(t_sb[:, 512 * m : 512 * (m + 1)], t_ps[:])
            for h in range(H):
                cps = ctx_ps[(j, h % 2)]
                q = h // 2
                for k in range(CK):
                    nc.tensor.matmul(
                        cps[:, 64 * q : 64 * (q + 1)],
                        wv(j, 1024 * k + 512 + 64 * h, 1024 * k + 512 + 64 * (h + 1)),
                        t_sb[:, 512 * k + 64 * h : 512 * k + 64 * (h + 1)],
                        start=(k == 0),
                        stop=(k == CK - 1),
                    )

        def softmax(j, ctx_ps):
            """Batched softmax: K and V share x, so ctx has a correlated mean
            component and scaled logits spread ~N(0,24) - max-subtraction is
            required (tails overflow f32 exp). Per-head maxes come from one
            3D reduce per parity; the subtract is a broadcast
            scalar_tensor_tensor; exp/sums/reciprocal are batched; fp16
            head-pairs go through PE pair transposes with in-lane quadrant
            copies into s_sb."""
            nmax = smp.tile([64, 8], F32, name="nmax", tag="nmax")
            nmaxs = smp.tile([64, 8], F32, name="nmaxs", tag="nmaxs")
            z_sb = smp.tile([64, 512], F32, name="zsb", tag="zsb")
            e_sb = smp.tile([64, 512], F32, name="esb", tag="esb")
            for par in range(2):
                nc.vector.tensor_reduce(
                    nmax[:, 4 * par : 4 * par + 4],
                    ctx_ps[(j, par)].rearrange("p (q c) -> p q c", c=64),
                    mybir.AxisListType.X, mybir.AluOpType.max, negate=True,
                )
            nc.vector.tensor_scalar_mul(nmaxs[:], nmax[:], SCALE)
            for par in range(2):
                nc.vector.scalar_tensor_tensor(
                    z_sb[:, 256 * par : 256 * (par + 1)].rearrange("p (q c) -> p q c", c=64),
                    ctx_ps[(j, par)].rearrange("p (q c) -> p q c", c=64),
                    SCALE,
                    nmaxs[:, 4 * par : 4 * par + 4].unsqueeze(2).broadcast_to([64, 4, 64]),
                    mybir.AluOpType.mult,
                    mybir.AluOpType.add,
                )
                nc.scalar.activation(
                    e_sb[:, 256 * par : 256 * (par + 1)],
                    z_sb[:, 256 * par : 256 * (par + 1)],
                    mybir.ActivationFunctionType.Exp,
                )
            ssum = smp.tile([64, 8], F32, name="ssum", tag="ssum")
            for par in range(2):
                nc.vector.tensor_reduce(
                    ssum[:, 4 * par : 4 * par + 4],
                    e_sb[:, 256 * par : 256 * (par + 1)].rearrange("p (q c) -> p q c", c=64),
                    mybir.AxisListType.X, mybir.AluOpType.add,
                )
            rec = smp.tile([64, 8], F32, name="rec", tag="rec")
            nc.vector.reciprocal(rec[:], ssum[:])
            # sT pair q: cols [128q:128q+64] = head 2q, [+64:+128] = head 2q+1
            sT = smp.tile([64, 512], F16, name="sT", tag="sT")
            for q in range(4):
                for par in range(2):
                    nc.vector.tensor_scalar_mul(
                        sT[:, 128 * q + 64 * par : 128 * q + 64 * (par + 1)],
                        e_sb[:, 256 * par + 64 * q : 256 * par + 64 * (q + 1)],
                        rec[:, 4 * par + q : 4 * par + q + 1],
                    )
                tp = scr_ps.tile([128, 64], F16, name="sps", tag="scr")
                nc.tensor.transpose(tp[:], sT[:, 128 * q : 128 * (q + 1)], ident16[0:64, 0:64])
                eng_copy(s_sb[0:64, j * 1024 + 128 * q : j * 1024 + 128 * q + 64], tp[0:64, :])
                eng_copy(s_sb[64:128, j * 1024 + 128 * q + 64 : j * 1024 + 128 * (q + 1)], tp[64:128, :])
            if j == 0 and "dbg_e" in io:
                nc.sync.dma_start(io["dbg_e"], e_sb[:])
                nc.sync.dma_start(io["dbg_sum"][:, 0:8], ssum[:])
                nc.sync.dma_start(io["dbg_sum"][:, 8:16], rec[:])
                nc.sync.dma_start(io["dbg_sT"], sT[:])

        opair = {}
        nstg = [0]

        def stage_copy(dst, src_):
            # Whole-tile PSUM->SBUF staging alternating DVE/ACT: combined
            # PSUM-read rate ~1.1TB/s keeps pace with the a-pass matmuls.
            nstg[0] += 1
            if nstg[0] % 2:
                nc.vector.tensor_copy(dst, src_)
            else:
                nc.scalar.activation(dst, src_, mybir.ActivationFunctionType.Copy)

        def emit_outT(j, k, g, o_ps):
            """Stage a finished outT tile (f32 PSUM -> fp16 SBUF); DMA every
            completed pair of token groups with one 256KB transfer. The final
            two groups of the very last a1 block go out singly so the drain
            tail is as short as possible."""
            if j == 0 and k == CK - 1 and g >= 6:
                o_sb = outp.tile([128, 1024], F16, name="osb", tag="osb")
                stage_copy(o_sb[:, 0:512], o_ps[:])
                nc.sync.dma_start(
                    o_d[j][128 * k : 128 * (k + 1), 512 * g : 512 * (g + 1)],
                    o_sb[:, 0:512],
                )
                return
            gg, half = divmod(g, 2)
            key = (j, k, gg)
            if key not in opair:
                opair[key] = outp.tile([128, 1024], F16, name="osb", tag="osb")
            o_sb = opair[key]
            c0 = 512 * half
            stage_copy(o_sb[:, c0 : c0 + 512], o_ps[:])
            if half == 1:
                o_sb = opair.pop(key)
                (nc.sync if j == 0 else nc.gpsimd).dma_start(
                    o_d[j][128 * k : 128 * (k + 1), 1024 * gg : 1024 * (gg + 1)],
                    o_sb[:, 0:1024],
                )

        def a_pass(jq, js, out_pool):
            """outT_{jq}[chout, tok] = sum_chin S_{js}[chin, chout] q_{jq}[tok, chin].
            s stationary, xT moving: 512 tokens per matmul."""
            for k in range(CK):
                for g in range(N // 512):
                    o_ps = out_pool.tile([128, 512], F32, name=f"o{jq}ps", tag="ops")
                    nc.tensor.matmul(
                        o_ps[:],
                        s_sb[:, js * 1024 + 128 * k : js * 1024 + 128 * (k + 1)],
                        xT_sb[:, jq * 16384 + k * 4096 + 512 * g : jq * 16384 + k * 4096 + 512 * (g + 1)],
                        start=True,
                        stop=True,
                    )
                    emit_outT(jq, k, g, o_ps)

        # ---------------- schedule ----------------
        ctx_ps = {}
        g1 = [big_ps.tile([128, 512], F32, name=f"g1{m}", tag=f"big{m}") for m in range(CK)]
        # Both W tensors gated just after the stream starts: they ride the
        # scalar queue 7-17us, done well before xT contention and in time
        # for the m-loops (a late W turns into multi-us engine-queue stalls
        # behind the wv16 casts).
        stream_g(0, g1, list(range(NG)),
                 gates={1: [wv(0, 1024 * k, 1024 * k + 1) for k in range(CK)],
                        3: [wv(1, 1024 * k, 1024 * k + 1) for k in range(CK)]})
        load_w(0)
        load_w(1)
        g_evac(0, g1)

        # G2 head groups cover the evac/mirror/t_ctx(0) seam on the PE: their
        # matmuls only WAR-wait on the g1 banks being copied out.
        g2 = [big_ps.tile([128, 512], F32, name=f"g2{m}", tag=f"big{m}") for m in range(CK)]
        stream_g(1, g2, list(range(0, 2)))

        mirror_t_ctx(0, ctx_ps)                              # T(1), ctxT(1)

        stream_g(1, g2, list(range(2, 4)),
                 gates={3: [xT_sb[:, 16384 + c * 4096 : 16385 + c * 4096] for c in range(CK)]})
        load_xT(1, nc.scalar)
        with tc.high_priority():
            softmax(0, ctx_ps)                               # s1
        stream_g(1, g2, list(range(4, NG)),
                 gates={5: [xT_sb[:, c * 4096 : c * 4096 + 1] for c in range(CK)]})
        load_xT(0, nc.sync)
        g_evac(1, g2)
        sc_big.close()

        mirror_t_ctx(1, ctx_ps)                              # T(2), ctxT(2)

        sc_out = ExitStack()
        out_pool = sc_out.enter_context(tc.tile_pool(name="out_ps", bufs=4, space="PSUM"))
        a_pass(1, 0, out_pool)                               # out2 = q2 @ s1
        with tc.high_priority():
            softmax(1, ctx_ps)                               # s2
        a_pass(0, 1, out_pool)                               # out1 = q1 @ s2
        sc_out.close()

        if "dbg_xT" in io:
            nc.sync.dma_start(io["dbg_xT"], xT_sb[:])
            nc.sync.dma_start(io["dbg_s"], s_sb[:])
            nc.sync.dma_start(io["dbg_g"], g_sb[:])


def _build():
    if "nc" in _CACHE:
        return _CACHE["nc"]
    nc = bacc.Bacc("TRN2", target_bir_lowering=False, debug=False, num_devices=B)
    io = {
        "x1": nc.dram_tensor("x1", [N, C], F16, kind="ExternalInput").ap(),
        "x2": nc.dram_tensor("x2", [N, C], F16, kind="ExternalInput").ap(),
        "x1T": nc.dram_tensor("x1T", [C, N], F16, kind="ExternalInput").ap(),
        "x2T": nc.dram_tensor("x2T", [C, N], F16, kind="ExternalInput").ap(),
        "Wkv1": nc.dram_tensor("Wkv1", [C, 2 * C], F16, kind="ExternalInput").ap(),
        "Wkv2": nc.dram_tensor("Wkv2", [C, 2 * C], F16, kind="ExternalInput").ap(),
        "out1": nc.dram_tensor("out1", [C, N], F16, kind="ExternalOutput").ap(),
        "out2": nc.dram_tensor("out2", [C, N], F16, kind="ExternalOutput").ap(),
    }

    with tile.TileContext(nc) as tc:
        _emit(tc, io)
    nc.compile()
    _CACHE["nc"] = nc
    return nc


def kernel(x1, x2, Wkv1, Wkv2):
    x1 = np.ascontiguousarray(np.asarray(x1, dtype=np.float32).astype(np.float16))
    x2 = np.ascontiguousarray(np.asarray(x2, dtype=np.float32).astype(np.float16))
    Wkv1 = np.ascontiguousarray(np.asarray(Wkv1, dtype=np.float32).astype(np.float16))
    Wkv2 = np.ascontiguousarray(np.asarray(Wkv2, dtype=np.float32).astype(np.float16))

    nc = _build()
    in_maps = [
        {
            "x1": x1[b], "x2": x2[b],
            "x1T": np.ascontiguousarray(x1[b].T),
            "x2T": np.ascontiguousarray(x2[b].T),
            "Wkv1": Wkv1, "Wkv2": Wkv2,
        }
        for b in range(B)
    ]
    res = run_bass_kernel_spmd(nc, in_maps, list(range(B))).results
    out1 = np.stack([res[b]["out1"].T for b in range(B)]).astype(np.float32)
    out2 = np.stack([res[b]["out2"].T for b in range(B)]).astype(np.float32)
    return out1, out2


if __name__ == "__main__":
    rng = np.random.default_rng(0)
    o1, o2 = kernel(
        rng.standard_normal((B, N, C), dtype=np.float32),
        rng.standard_normal((B, N, C), dtype=np.float32),
        rng.standard_normal((C, 2 * C), dtype=np.float32) * C**-0.5,
        rng.standard_normal((C, 2 * C), dtype=np.float32) * C**-0.5,
    )
    print(o1.shape, o2.shape)
